# revision 45
# baseline (speedup 1.0000x reference)
"""Multi-head self-attention (B=1, S=2048, E=1024, H=16, D=64) on 8 NeuronCores.

Tensor-parallel by head: core c owns heads {2c, 2c+1}.  v7 schedule:

  Phase A: 8 warm matmuls (HAM ramp); wk/wq + the first S-half of qkvT
    ride the two HW DGE queues (k-pair transfers); the second S-half is
    issued on gpsimd behind a data-gate so it never steals DMA bandwidth
    from the critical path; priority matmuls (kT c0, qT c0+c1) run
    k-outer as the pairs land.
  Phase B (t=0..15): scoresT(p0) per head (row-group concurrent K=64
    pairs), software-pipelined one t ahead of the exp stream so backlog
    bursts (rest of in_proj) never delay the exps.  h0's exp is exact on
    ACT; h1's is a one-pass Schraudolph bf16 exp on DVE (int16 bitcast,
    ~2% sawtooth that largely cancels in the softmax ratio).  AV c0
    trails.
  Phase C (t=0..15): scoresT(p1) + exp, same split.  AV c1 runs t=0..3
    (its exp data all exists at the seam); chunks c0/c1 are normalized
    in-phase (bit-trick + Newton 1/Z on DVE — divide is not an ISA op,
    ACT tables would thrash); out_proj tiles for c0/c1 stream through
    the phase with evictions (ACT/DVE) + output DMA.  AV c2 from t=4,
    AV c3 from t=10.
  Tail: only c2/c3: ACT table reciprocal (one switch), 8 out_proj
    tiles, evictions rotated over scalar/vector, output DMA striped.

Host sums the 8 partials and adds b_out.
"""

import os
import sys

import numpy as np

try:
    import concourse.bass as bass  # noqa: F401
except ImportError:
    sys.path.insert(0, "/opt/trn_rl_repo")

import ml_dtypes

import concourse.bass as bass
import concourse.mybir as mybir
import concourse.tile as tile
from concourse import bacc, bass_utils

S = 2048
E = 1024
H = 16
D = 64
NCORE = 8
HC = H // NCORE          # heads per core = 2
J = HC * D               # local feature width = 128
KE = E // 128            # contraction tiles for in_proj = 8
NT = S // 128            # 128-row tiles of the sequence = 16
CH = 512                 # s-chunk (one PSUM bank of f32)
NCH = S // CH            # s-chunks = 4
W = 2 * CH               # score tile width (2 chunks) = 1024
SCALE = 1.0 / np.sqrt(D)

MM_DT = mybir.dt.bfloat16
MM_NP = ml_dtypes.bfloat16

_cached = None


def _build():
    f32 = mybir.dt.float32
    f16 = mybir.dt.float16
    div = mybir.AluOpType.divide
    nc = bacc.Bacc("TRN2", target_bir_lowering=False, num_swdge_queues=4)

    d_qkvT = nc.dram_tensor("qkvT", (E, S), MM_DT, kind="ExternalInput")
    d_wq = nc.dram_tensor("wq", (128, KE * J), MM_DT, kind="ExternalInput")
    d_wk = nc.dram_tensor("wk", (128, KE * J), MM_DT, kind="ExternalInput")
    d_wv = nc.dram_tensor("wv", (128, KE * J), MM_DT, kind="ExternalInput")
    d_bq = nc.dram_tensor("bq", (J, 1), f32, kind="ExternalInput")
    d_bk = nc.dram_tensor("bk", (J, 1), f32, kind="ExternalInput")
    d_bv = nc.dram_tensor("bv", (128, J), f32, kind="ExternalInput")
    d_wout = nc.dram_tensor("wout", (J, E), MM_DT, kind="ExternalInput")
    d_out = nc.dram_tensor("partial", (S, E), f16, kind="ExternalOutput")

    dq = d_qkvT.rearrange("(k p) m -> p k m", p=128)

    with tile.TileContext(nc) as tc:
        with (
            tc.tile_pool(name="persist", bufs=1) as persist,
            tc.tile_pool(name="outp", bufs=8) as outp,
            tc.tile_pool(name="small", bufs=4) as small,
            # PSUM: 2 x [128,1024] (scores / qT pair)              = 4 banks
            #       4 x [128,512]  (AV + in_proj + out_proj accum) = 4 banks
            tc.tile_pool(name="ps_sc", bufs=2, space="PSUM") as ps_sc,
            tc.tile_pool(name="ps_a", bufs=4, space="PSUM") as ps_a,
        ):
            # ---- persistent SBUF ----
            sb_wq = persist.tile([128, KE, J], MM_DT)
            sb_wk = persist.tile([128, KE, J], MM_DT)
            sb_wv = persist.tile([128, KE, J], MM_DT)
            sb_bq = persist.tile([J, 1], f32)
            sb_bk = persist.tile([J, 1], f32)
            sb_bv = persist.tile([128, HC, D], f32)
            sb_bv2 = sb_bv
            sb_wout = persist.tile([J, E], MM_DT)
            sb_qkvT = persist.tile([128, KE, S], MM_DT)
            sb_qT = persist.tile([J, S], MM_DT)
            sb_kT = persist.tile([J, S], MM_DT)
            # v augmented per head with a 64-wide ones block: the AV matmul
            # then yields Z replicated on partitions 64..127
            sb_v = persist.tile([128, NT, HC, 2, D], MM_DT)
            sb_attnT = persist.tile([J, S], MM_DT)
            # exp buffers, one per head; C-phase aliases the same storage
            ex = [persist.tile([128, NT, W], MM_DT, name=f"ex{h}") for h in range(HC)]
            sb_warm = persist.tile([128, CH], MM_DT)

            # ---- t0: memsets (vector/gpsimd) ----
            nc.vector.memset(sb_warm[:], 0.125)
            nc.vector.memset(sb_v[:, :, 0, 1, :], 1.0)
            nc.gpsimd.memset(sb_v[:, :, 1, 1, :], 1.0)

            # ---- DMA issues ----
            # Everything the priority chain needs goes first (wk, wq, the
            # first S-half of qkvT) — descriptors of all in-flight
            # transfers share the 16 DMA engines, so the second half is
            # issued from the VECTOR queue *after* the priority bias adds:
            # it then starts streaming only once the first half is done
            # and never competes with the critical path.
            nc.sync.dma_start(out=sb_wk[:], in_=d_wk[:])
            nc.scalar.dma_start(out=sb_wq[:], in_=d_wq[:])
            hw2 = [nc.sync, nc.scalar]
            for kk in range(4):  # first half, k-pair granularity for gating
                hw2[kk % 2].dma_start(
                    out=sb_qkvT[:, 2 * kk : 2 * kk + 2, 0:1024],
                    in_=dq[:, 2 * kk : 2 * kk + 2, 0:1024],
                )
            nc.gpsimd.dma_start(out=sb_bq[:], in_=d_bq[:])
            nc.gpsimd.dma_start(out=sb_bk[:], in_=d_bk[:])
            nc.gpsimd.dma_start(out=sb_wv[:], in_=d_wv[:])
            nc.gpsimd.dma_start(out=sb_bv[:], in_=d_bv[:])

            # ---- phase A: priority in_proj (kT c0, qT c0+c1), k-outer,
            # warm fillers interleaved so HAM stays hot through the
            # DMA-gated waits.  Warm tiles live in the ps_a ring so they
            # don't cycle with q01 in the 2-deep sc ring. ----
            warm_ps = [
                ps_a.tile([128, CH], f32, tag="a", name=f"warm{i}")
                for i in range(2)
            ]

            def warm(n):
                for i in range(n):
                    nc.tensor.matmul(
                        warm_ps[i % 2][:], sb_warm[:, 0:128], sb_warm[:],
                        start=True, stop=True,
                    )

            warm(8)
            ps_kc0 = ps_a.tile([128, CH], f32, tag="a", name="kc0")
            ps_q01 = ps_sc.tile([128, W], f32, tag="sc", name="q01")
            for k in range(KE):
                nc.tensor.matmul(
                    ps_kc0[:], sb_wk[:, k, :], sb_qkvT[:, k, 0:CH],
                    start=(k == 0), stop=(k == KE - 1),
                )
                nc.tensor.matmul(
                    ps_q01[:, 0:CH], sb_wq[:, k, :], sb_qkvT[:, k, 0:CH],
                    start=(k == 0), stop=(k == KE - 1),
                )
                nc.tensor.matmul(
                    ps_q01[:, CH:W], sb_wq[:, k, :], sb_qkvT[:, k, CH:W],
                    start=(k == 0), stop=(k == KE - 1),
                )

            nc.vector.tensor_scalar_add(sb_kT[:, 0:CH], ps_kc0[:], sb_bk[:])
            nc.vector.tensor_scalar_add(sb_qT[:, 0:W], ps_q01[:], sb_bq[:])
            # second S-half + wout: issued on gpsimd BEHIND a tiny copy
            # that depends on the priority-add output, so these transfers
            # only start streaming once the first half has landed and
            # never steal DMA bandwidth from the critical path.
            nc.gpsimd.tensor_copy(sb_warm[0:1, 0:1], sb_qT[0:1, 0:1])
            nc.gpsimd.dma_start(
                out=sb_qkvT[:, 0:4, 1024:2048], in_=dq[:, 0:4, 1024:2048]
            )
            nc.gpsimd.dma_start(
                out=sb_qkvT[:, 4:8, 1024:2048], in_=dq[:, 4:8, 1024:2048]
            )
            nc.gpsimd.dma_start(out=sb_wout[:], in_=d_wout[:])

            # ---- backlog filler units for phase B ----
            def mk_vgroup(g, eng):
                def emit():
                    ps_v = ps_a.tile([128, 4, HC, D], f32, tag="a", name=f"v{g}")
                    for ti in range(4):
                        t = 4 * g + ti
                        for k in range(KE):
                            nc.tensor.matmul(
                                ps_v[:, ti, :, :],
                                sb_qkvT[:, k, t * 128 : (t + 1) * 128],
                                sb_wv[:, k, :],
                                start=(k == 0), stop=(k == KE - 1),
                            )
                    # add bias into sb_v (v cols only, skip the ones
                    # blocks) — one strided add per t-tile
                    for ti in range(4):
                        t = 4 * g + ti
                        eng.tensor_add(
                            sb_v[:, t, :, 0, :], ps_v[:, ti, :, :], sb_bv2[:]
                        )
                return emit

            def mk_kchunk(c, eng):
                def emit():
                    ps_k = ps_a.tile([128, CH], f32, tag="a", name=f"kc{c}")
                    for k in range(KE):
                        nc.tensor.matmul(
                            ps_k[:], sb_wk[:, k, :],
                            sb_qkvT[:, k, c * CH : (c + 1) * CH],
                            start=(k == 0), stop=(k == KE - 1),
                        )
                    eng.tensor_scalar_add(
                        sb_kT[:, c * CH : (c + 1) * CH], ps_k[:], sb_bk[:]
                    )
                return emit

            def mk_qchunk(c, eng):
                def emit():
                    ps_q = ps_a.tile([128, CH], f32, tag="a", name=f"qc{c}")
                    for k in range(KE):
                        nc.tensor.matmul(
                            ps_q[:], sb_wq[:, k, :],
                            sb_qkvT[:, k, c * CH : (c + 1) * CH],
                            start=(k == 0), stop=(k == KE - 1),
                        )
                    eng.tensor_scalar_add(
                        sb_qT[:, c * CH : (c + 1) * CH], ps_q[:], sb_bq[:]
                    )
                return emit

            # emitted at END of B-iteration t (key); consumers:
            # kT chunk c needed by scores at t=4c; v group g by av_c0 per
            # av0_steps; qT c2/c3 by phase C.
            backlog = {
                0: mk_vgroup(0, nc.vector),
                2: mk_kchunk(1, nc.vector),
                4: mk_vgroup(1, nc.vector),
                5: mk_kchunk(2, nc.vector),
                6: mk_vgroup(2, nc.vector),
                9: mk_kchunk(3, nc.vector),
                10: mk_qchunk(2, nc.vector),
                12: mk_vgroup(3, nc.vector),
                13: mk_qchunk(3, nc.vector),
            }

            def scores(t, h, qlo, ps):
                hd = slice(h * D, (h + 1) * D)
                for i in range(2):
                    nc.tensor.matmul(
                        ps[:, i * CH : (i + 1) * CH],
                        sb_kT[hd, t * 128 : (t + 1) * 128],
                        sb_qT[hd, qlo + i * CH : qlo + (i + 1) * CH],
                        start=True, stop=True,
                    )

            def av_step(slot, h, chalf, t, first, last):
                nc.tensor.matmul(
                    slot[:],
                    sb_v[:, t, h, :, :],
                    ex[h][:, t, chalf * CH : (chalf + 1) * CH],
                    start=first, stop=last,
                )

            RECIP_MAGIC = float(0x7EF127EA)
            i32 = mybir.dt.int32
            mult = mybir.AluOpType.mult
            addop = mybir.AluOpType.add

            def normalize(c, avx, heads=(0, 1)):
                # 1/Z on DVE (no ACT table swap): Schraudolph bit-trick
                # seed + 1 Newton step, then attnT[hd, chunk] = av_out*rz.
                # (divide is not an ISA op; GPSIMD cannot touch PSUM.)
                s_sl = slice(c * CH, (c + 1) * CH)
                for h in heads:
                    hd = slice(h * D, (h + 1) * D)
                    z_ap = avx[h][D : 2 * D, :]
                    rz = small.tile([D, CH], f32, tag="rz", name=f"rz{c}_{h}")
                    u = small.tile([D, CH], f32, tag="u", name=f"u{c}_{h}")
                    nc.vector.tensor_scalar(
                        rz[:].bitcast(i32), z_ap.bitcast(i32), -1, RECIP_MAGIC,
                        mult, addop,
                    )
                    for _ in range(1):
                        nc.vector.tensor_tensor(u[:], z_ap, rz[:], mult)
                        nc.vector.tensor_scalar(u[:], u[:], -1.0, 2.0, mult, addop)
                        nc.vector.tensor_tensor(rz[:], rz[:], u[:], mult)
                    nc.vector.tensor_mul(sb_attnT[hd, s_sl], avx[h][0:D, :], rz[:])

            def evict(eng, dst, src):
                if eng is nc.scalar:
                    eng.copy(dst, src)
                else:
                    eng.tensor_copy(dst, src)

            # out_proj for one 128-row s-tile: 2 half matmuls + evictions
            # + output DMA.  evict_engs/dma_eng chosen per call site.
            def out_tile(ts, evict_engs, dma_eng):
                sb_out = outp.tile([128, E], f16, tag="out", name=f"o{ts}")
                for ec in range(2):
                    ps_p = ps_a.tile([128, CH], f32, tag="a", name=f"op{ts}_{ec}")
                    nc.tensor.matmul(
                        ps_p[:],
                        sb_attnT[:, ts * 128 : (ts + 1) * 128],
                        sb_wout[:, ec * CH : (ec + 1) * CH],
                        start=True, stop=True,
                    )
                    evict(evict_engs[ec], sb_out[:, ec * CH : (ec + 1) * CH], ps_p[:])
                dma_eng.dma_start(
                    out=d_out[ts * 128 : (ts + 1) * 128, :], in_=sb_out[:]
                )

            # ---- phase B: scores p0 + exp; filler: backlog + AV c0 ----
            av0 = [ps_a.tile([128, CH], f32, tag="a", name=f"av0_{h}") for h in range(HC)]
            # av_c0 pacing: 16 steps over t=3..15 -> [2,1,1,1,1,1,1,1,1,1,1,2,2]
            av0_steps = {t: [] for t in range(NT)}
            j = 0
            for t in range(3, NT):
                n = 2 if t in (3, 14, 15) else 1
                for _ in range(n):
                    if j < NT:
                        av0_steps[t].append(j)
                        j += 1
            # scores run one t ahead of the exp stream: scores(t+1, h) is
            # emitted right after exp(t, h) so it starts the moment the
            # sc-ring slot frees — backlog/AV bursts never delay the exps.
            def mk_scores(phase, t, qlo):
                sc_ps = [
                    ps_sc.tile([128, W], f32, tag="sc", name=f"sc{phase}{t}_{h}")
                    for h in range(HC)
                ]
                for h in range(HC):
                    scores(t, h, qlo, sc_ps[h])
                return sc_ps

            # h0's exp runs exact on ACT; h1's runs as a one-pass
            # Schraudolph bf16 exp on DVE (bits = round(x*SCALE*128/ln2 +
            # 16251.5) read as bf16, ~2% sawtooth that largely cancels in
            # the softmax ratio) — halves the ACT stream, the old pacer.
            EXPA = float(SCALE * 128.0 / np.log(2.0))
            EXPB = 16251.5
            i16 = mybir.dt.int16

            def mk_exp(sc_ps, t):
                nc.scalar.activation(
                    ex[0][:, t, :], sc_ps[0][:],
                    mybir.ActivationFunctionType.Exp, scale=float(SCALE),
                )
                nc.vector.tensor_scalar(
                    ex[1][:, t, :].bitcast(i16), sc_ps[1][:], EXPA, EXPB,
                    mult, addop,
                )

            sc_cur = mk_scores("B", 0, 0)
            for t in range(NT):
                mk_exp(sc_cur, t)
                if t + 1 < NT:
                    sc_cur = mk_scores("B", t + 1, 0)
                for jj in av0_steps[t]:
                    for h in range(HC):
                        av_step(av0[h], h, 0, jj, jj == 0, jj == NT - 1)
                if t in backlog:
                    backlog[t]()

            # ---- B seam: normalize chunk c0 h0; h1's chain runs at C t=1
            # so it doesn't collide with the first offloaded exp ----
            normalize(0, av0, heads=(0,))

            # ---- phase C: scores p1 + exp; av1 t=0..3, av2 t>=4, av3 t>=9;
            #      out_proj c0 tiles t=1..4, c1 tiles t=5..8 ----
            av1 = [ps_a.tile([128, CH], f32, tag="a", name=f"av1_{h}") for h in range(HC)]
            av2 = None
            av3 = None
            # av1 consumes only B-phase ex values: run 4 steps/t for t=0..3
            # (stays ahead of the exp aliasing overwrite of ex[:, t, CH:W])
            av2_steps = {t: [] for t in range(NT)}
            j = 0
            for t in range(4, NT):
                n = {4: 2, 5: 2, 6: 2, 7: 2}.get(t, 1)
                for _ in range(n):
                    if j < NT:
                        av2_steps[t].append(j)
                        j += 1
            av3_steps = {t: [] for t in range(NT)}
            j = 0
            for t in range(10, NT):
                n = {15: 1}.get(t, 3)
                for _ in range(n):
                    if j < NT:
                        av3_steps[t].append(j)
                        j += 1
            # av1 j=0 must be read before exp_C(0) overwrites ex[:,0,:]
            # (Tile orders the WAR either way; emitting it first keeps the
            # exp from waiting long).
            for h in range(HC):
                av_step(av1[h], h, 1, 0, True, False)
            sc_cur = mk_scores("C", 0, W)
            for t in range(NT):
                mk_exp(sc_cur, t)
                if t + 1 < NT:
                    sc_cur = mk_scores("C", t + 1, W)
                if t < 4:
                    for jj in range(4 * t, 4 * t + 4):
                        if t == 0 and jj == 0:
                            continue
                        for h in range(HC):
                            av_step(av1[h], h, 1, jj, jj == 0, jj == NT - 1)
                if t == 1:
                    normalize(0, av0, heads=(1,))
                if t == 4:
                    # av1 complete: normalize chunk c1, freeing its banks
                    normalize(1, av1, heads=(0,))
                    av2 = [
                        ps_a.tile([128, CH], f32, tag="a", name=f"av2_{h}")
                        for h in range(HC)
                    ]
                if t == 5:
                    normalize(1, av1, heads=(1,))
                if t == 10:
                    av3 = [
                        ps_a.tile([128, CH], f32, tag="a", name=f"av3_{h}")
                        for h in range(HC)
                    ]
                for jj in av2_steps[t]:
                    for h in range(HC):
                        av_step(av2[h], h, 0, jj, jj == 0, jj == NT - 1)
                for jj in av3_steps[t]:
                    for h in range(HC):
                        av_step(av3[h], h, 1, jj, jj == 0, jj == NT - 1)
                # out_proj stream for the first-half chunks; ec0 eviction
                # rides ACT's slack (only h0 exps run there now)
                if 2 <= t <= 5:
                    out_tile(t - 2, (nc.scalar, nc.vector), nc.sync)
                elif 6 <= t <= 9:
                    out_tile(t - 2, (nc.scalar, nc.vector), nc.gpsimd)

            # ---- tail: c2/c3 normalize + out_proj + evict + DMA out ----
            # ACT is idle after the last exp: use table reciprocal there
            # (one table switch), muls on DVE.
            def act_recip(out_ap, in_ap):
                eng = nc.scalar
                inst = mybir.InstActivation(
                    name=nc.get_next_instruction_name(),
                    func=mybir.ActivationFunctionType.Reciprocal,
                    ins=[
                        eng.lower_ap(in_ap),
                        mybir.ImmediateValue(dtype=f32, value=0.0),
                        mybir.ImmediateValue(dtype=f32, value=1.0),
                        mybir.ImmediateValue(dtype=f32, value=0.0),
                    ],
                    outs=[eng.lower_ap(out_ap)],
                )
                eng.add_instruction(inst)

            for c, avx in ((2, av2), (3, av3)):
                s_sl = slice(c * CH, (c + 1) * CH)
                for h in range(HC):
                    hd = slice(h * D, (h + 1) * D)
                    rbc = small.tile([D, CH], f32, tag="rz", name=f"rbc{c}_{h}")
                    act_recip(rbc[:], avx[h][D : 2 * D, :])
                    nc.vector.tensor_mul(sb_attnT[hd, s_sl], avx[h][0:D, :], rbc[:])
            evict_rot = [
                (nc.scalar, nc.vector), (nc.vector, nc.scalar),
                (nc.scalar, nc.vector), (nc.vector, nc.scalar),
                (nc.scalar, nc.vector), (nc.vector, nc.scalar),
                (nc.scalar, nc.vector), (nc.vector, nc.scalar),
            ]
            dma_rot = [nc.sync, nc.gpsimd, nc.scalar, nc.sync,
                       nc.gpsimd, nc.scalar, nc.sync, nc.gpsimd]
            for i, ts in enumerate(range(8, 16)):
                out_tile(ts, evict_rot[i], dma_rot[i])

    nc.finalize()
    return nc


def _pack_w(w):
    # [E, J] -> [128, KE*J] in (p, k, m) order for a contiguous-row DMA
    return np.ascontiguousarray(
        np.asarray(w, np.float32).reshape(KE, 128, J).transpose(1, 0, 2).reshape(128, KE * J)
    ).astype(MM_NP)


def _prep_inputs(qkv, w_in, b_in, w_out):
    qkv2 = np.asarray(qkv, np.float32).reshape(S, E)
    qkvT = np.ascontiguousarray(qkv2.T).astype(MM_NP)
    w_in = np.asarray(w_in, np.float32)
    b_in = np.asarray(b_in, np.float32)
    w_out = np.asarray(w_out, np.float32)
    in_maps = []
    for c in range(NCORE):
        cols = slice(c * J, c * J + J)
        in_maps.append(
            {
                "qkvT": qkvT,
                "wq": _pack_w(w_in[:, :E][:, cols]),
                "wk": _pack_w(w_in[:, E : 2 * E][:, cols]),
                "wv": _pack_w(w_in[:, 2 * E :][:, cols]),
                "bq": np.ascontiguousarray(b_in[:E][cols]).reshape(J, 1),
                "bk": np.ascontiguousarray(b_in[E : 2 * E][cols]).reshape(J, 1),
                "bv": np.broadcast_to(
                    b_in[2 * E :][cols].reshape(1, J), (128, J)
                ).copy(),
                "wout": np.ascontiguousarray(w_out[cols, :]).astype(MM_NP),
            }
        )
    return in_maps


def kernel(qkv, w_in, b_in, w_out, b_out, _trace=False):
    global _cached
    if _cached is None:
        _cached = _build()
    nc = _cached
    in_maps = _prep_inputs(qkv, w_in, b_in, w_out)
    res = bass_utils.run_bass_kernel_spmd(
        nc, in_maps, core_ids=list(range(NCORE)), trace=_trace
    )
    acc = np.zeros((S, E), np.float64)
    for r in res.results:
        acc += r["partial"].astype(np.float64)
    out = (acc + np.asarray(b_out, np.float32)[None, :]).astype(np.float32)
    out = out.reshape(1, S, E)
    if _trace:
        kernel.last_exec_time_ns = res.exec_time_ns
    return out


# revision 53
# speedup vs baseline: 1.0346x; 1.0346x over previous
"""Multi-head self-attention (B=1, S=2048, E=1024, H=16, D=64) on 8 NeuronCores.

Tensor-parallel by head: core c owns heads {2c, 2c+1}.  v7 schedule:

  Phase A: 8 warm matmuls (HAM ramp); wk/wq + the first S-half of qkvT
    ride the two HW DGE queues (k-pair transfers); the second S-half is
    issued on gpsimd behind a data-gate so it never steals DMA bandwidth
    from the critical path; priority matmuls (kT c0, qT c0+c1) run
    k-outer as the pairs land.
  Phase B (t=0..15): scoresT(p0) per head (row-group concurrent K=64
    pairs), software-pipelined one t ahead of the exp stream so backlog
    bursts (rest of in_proj) never delay the exps.  h0's exp is exact on
    ACT; h1's is a one-pass Schraudolph bf16 exp on DVE (int16 bitcast,
    ~2% sawtooth that largely cancels in the softmax ratio).  AV c0
    trails.
  Phase C (t=0..15): scoresT(p1) + exp, same split.  AV c1 runs t=0..3
    (its exp data all exists at the seam); chunks c0/c1 are normalized
    in-phase (bit-trick + Newton 1/Z on DVE — divide is not an ISA op,
    ACT tables would thrash); out_proj tiles for c0/c1 stream through
    the phase with evictions (ACT/DVE) + output DMA.  AV c2 from t=4,
    AV c3 from t=10.
  Tail: only c2/c3: ACT table reciprocal (one switch), 8 out_proj
    tiles, evictions rotated over scalar/vector, output DMA striped.

Host sums the 8 partials and adds b_out.
"""

import os
import sys

import numpy as np

try:
    import concourse.bass as bass  # noqa: F401
except ImportError:
    sys.path.insert(0, "/opt/trn_rl_repo")

import ml_dtypes

import concourse.bass as bass
import concourse.mybir as mybir
import concourse.tile as tile
from concourse import bacc, bass_utils

S = 2048
E = 1024
H = 16
D = 64
NCORE = 8
HC = H // NCORE          # heads per core = 2
J = HC * D               # local feature width = 128
KE = E // 128            # contraction tiles for in_proj = 8
NT = S // 128            # 128-row tiles of the sequence = 16
CH = 512                 # s-chunk (one PSUM bank of f32)
NCH = S // CH            # s-chunks = 4
W = 2 * CH               # score tile width (2 chunks) = 1024
SCALE = 1.0 / np.sqrt(D)

MM_DT = mybir.dt.bfloat16
MM_NP = ml_dtypes.bfloat16

_cached = None


def _build():
    f32 = mybir.dt.float32
    f16 = mybir.dt.float16
    div = mybir.AluOpType.divide
    nc = bacc.Bacc("TRN2", target_bir_lowering=False, num_swdge_queues=4)

    d_qkvT = nc.dram_tensor("qkvT", (E, S), MM_DT, kind="ExternalInput")
    d_wq = nc.dram_tensor("wq", (128, KE * J), MM_DT, kind="ExternalInput")
    d_wk = nc.dram_tensor("wk", (128, KE * J), MM_DT, kind="ExternalInput")
    d_wv = nc.dram_tensor("wv", (128, KE * J), MM_DT, kind="ExternalInput")
    d_bq = nc.dram_tensor("bq", (J, 1), f32, kind="ExternalInput")
    d_bk = nc.dram_tensor("bk", (J, 1), f32, kind="ExternalInput")
    d_bv = nc.dram_tensor("bv", (128, J), f32, kind="ExternalInput")
    d_wout = nc.dram_tensor("wout", (J, E), MM_DT, kind="ExternalInput")
    d_out = nc.dram_tensor("partial", (S, E), f16, kind="ExternalOutput")

    dq = d_qkvT.rearrange("(k p) m -> p k m", p=128)

    with tile.TileContext(nc) as tc:
        with (
            tc.tile_pool(name="persist", bufs=1) as persist,
            tc.tile_pool(name="outp", bufs=8) as outp,
            tc.tile_pool(name="small", bufs=4) as small,
            # PSUM: 2 x [128,1024] (scores / qT pair)              = 4 banks
            #       4 x [128,512]  (AV + in_proj + out_proj accum) = 4 banks
            tc.tile_pool(name="ps_sc", bufs=2, space="PSUM") as ps_sc,
            tc.tile_pool(name="ps_a", bufs=4, space="PSUM") as ps_a,
        ):
            # ---- persistent SBUF ----
            sb_wq = persist.tile([128, KE, J], MM_DT)
            sb_wk = persist.tile([128, KE, J], MM_DT)
            sb_wv = persist.tile([128, KE, J], MM_DT)
            sb_bq = persist.tile([J, 1], f32)
            sb_bk = persist.tile([J, 1], f32)
            sb_bv = persist.tile([128, HC, D], f32)
            sb_bv2 = sb_bv
            sb_wout = persist.tile([J, E], MM_DT)
            sb_qkvT = persist.tile([128, KE, S], MM_DT)
            sb_qT = persist.tile([J, S], MM_DT)
            sb_kT = persist.tile([J, S], MM_DT)
            # v augmented per head with a 64-wide ones block: the AV matmul
            # then yields Z replicated on partitions 64..127
            sb_v = persist.tile([128, NT, HC, 2, D], MM_DT)
            sb_attnT = persist.tile([J, S], MM_DT)
            # exp buffers, one per head; C-phase aliases the same storage
            ex = [persist.tile([128, NT, W], MM_DT, name=f"ex{h}") for h in range(HC)]
            sb_warm = persist.tile([128, CH], MM_DT)

            # ---- t0: memsets (vector/gpsimd) ----
            nc.vector.memset(sb_warm[:], 0.125)
            nc.vector.memset(sb_v[:, :, 0, 1, :], 1.0)
            nc.gpsimd.memset(sb_v[:, :, 1, 1, :], 1.0)

            # ---- DMA issues ----
            # Everything the priority chain needs goes first (wk, wq, the
            # first S-half of qkvT) — descriptors of all in-flight
            # transfers share the 16 DMA engines, so the second half is
            # issued from the VECTOR queue *after* the priority bias adds:
            # it then starts streaming only once the first half is done
            # and never competes with the critical path.
            nc.sync.dma_start(out=sb_wk[:], in_=d_wk[:])
            nc.scalar.dma_start(out=sb_wq[:], in_=d_wq[:])
            hw2 = [nc.sync, nc.scalar]
            for kk in range(4):  # first half, k-pair granularity for gating
                hw2[kk % 2].dma_start(
                    out=sb_qkvT[:, 2 * kk : 2 * kk + 2, 0:1024],
                    in_=dq[:, 2 * kk : 2 * kk + 2, 0:1024],
                )
            nc.gpsimd.dma_start(out=sb_bq[:], in_=d_bq[:])
            nc.gpsimd.dma_start(out=sb_bk[:], in_=d_bk[:])
            nc.gpsimd.dma_start(out=sb_wv[:], in_=d_wv[:])
            nc.gpsimd.dma_start(out=sb_bv[:], in_=d_bv[:])

            # ---- phase A: priority in_proj (kT c0, qT c0+c1), k-outer,
            # warm fillers interleaved so HAM stays hot through the
            # DMA-gated waits.  Warm tiles live in the ps_a ring so they
            # don't cycle with q01 in the 2-deep sc ring. ----
            warm_ps = [
                ps_a.tile([128, CH], f32, tag="a", name=f"warm{i}")
                for i in range(2)
            ]

            def warm(n):
                for i in range(n):
                    nc.tensor.matmul(
                        warm_ps[i % 2][:], sb_warm[:, 0:128], sb_warm[:],
                        start=True, stop=True,
                    )

            warm(8)
            ps_kc0 = ps_a.tile([128, CH], f32, tag="a", name="kc0")
            ps_q01 = ps_sc.tile([128, W], f32, tag="sc", name="q01")
            for k in range(KE):
                nc.tensor.matmul(
                    ps_kc0[:], sb_wk[:, k, :], sb_qkvT[:, k, 0:CH],
                    start=(k == 0), stop=(k == KE - 1),
                )
                nc.tensor.matmul(
                    ps_q01[:, 0:CH], sb_wq[:, k, :], sb_qkvT[:, k, 0:CH],
                    start=(k == 0), stop=(k == KE - 1),
                )
                nc.tensor.matmul(
                    ps_q01[:, CH:W], sb_wq[:, k, :], sb_qkvT[:, k, CH:W],
                    start=(k == 0), stop=(k == KE - 1),
                )

            # bias adds are per-partition-bias ops: ACT's activation(bias=)
            # form handles them, running parallel to DVE (identity is in
            # every ACT table, so no table-load cost)
            def bias_add_act(out, in_, bias):
                nc.scalar.activation(
                    out, in_, mybir.ActivationFunctionType.Identity,
                    bias=bias, scale=1.0,
                )

            bias_add_act(sb_kT[:, 0:CH], ps_kc0[:], sb_bk[:])
            bias_add_act(sb_qT[:, 0:CH], ps_q01[:, 0:CH], sb_bq[:])
            nc.vector.tensor_scalar_add(sb_qT[:, CH:W], ps_q01[:, CH:W], sb_bq[:])
            # second S-half + wout: issued on gpsimd BEHIND a tiny copy
            # that depends on the priority-add output, so these transfers
            # only start streaming once the first half has landed and
            # never steal DMA bandwidth from the critical path.
            nc.gpsimd.tensor_copy(sb_warm[0:1, 0:1], sb_qT[0:1, 0:1])
            nc.gpsimd.dma_start(
                out=sb_qkvT[:, 0:4, 1024:2048], in_=dq[:, 0:4, 1024:2048]
            )
            nc.gpsimd.dma_start(
                out=sb_qkvT[:, 4:8, 1024:2048], in_=dq[:, 4:8, 1024:2048]
            )
            nc.gpsimd.dma_start(out=sb_wout[:], in_=d_wout[:])

            # ---- backlog filler units for phase B ----
            def mk_vgroup(g, eng):
                def emit():
                    ps_v = ps_a.tile([128, 4, HC, D], f32, tag="a", name=f"v{g}")
                    for ti in range(4):
                        t = 4 * g + ti
                        for k in range(KE):
                            nc.tensor.matmul(
                                ps_v[:, ti, :, :],
                                sb_qkvT[:, k, t * 128 : (t + 1) * 128],
                                sb_wv[:, k, :],
                                start=(k == 0), stop=(k == KE - 1),
                            )
                    # add bias into sb_v (v cols only, skip the ones
                    # blocks) — one strided add per t-tile
                    for ti in range(4):
                        t = 4 * g + ti
                        eng.tensor_add(
                            sb_v[:, t, :, 0, :], ps_v[:, ti, :, :], sb_bv2[:]
                        )
                return emit

            # kchunk/qchunk adds run on ACT, but DEFERRED one iteration so
            # they never head-of-line-block the exp stream while their
            # matmul accumulation is still in flight.
            deferred_adds = {t: [] for t in range(NT + 1)}

            def mk_kchunk(c, eng, t_add):
                def emit():
                    ps_k = ps_a.tile([128, CH], f32, tag="a", name=f"kc{c}")
                    for k in range(KE):
                        nc.tensor.matmul(
                            ps_k[:], sb_wk[:, k, :],
                            sb_qkvT[:, k, c * CH : (c + 1) * CH],
                            start=(k == 0), stop=(k == KE - 1),
                        )
                    deferred_adds[t_add].append(
                        lambda: bias_add_act(
                            sb_kT[:, c * CH : (c + 1) * CH], ps_k[:], sb_bk[:]
                        )
                    )
                return emit

            def mk_qchunk(c, eng, t_add):
                def emit():
                    ps_q = ps_a.tile([128, CH], f32, tag="a", name=f"qc{c}")
                    for k in range(KE):
                        nc.tensor.matmul(
                            ps_q[:], sb_wq[:, k, :],
                            sb_qkvT[:, k, c * CH : (c + 1) * CH],
                            start=(k == 0), stop=(k == KE - 1),
                        )
                    deferred_adds[t_add].append(
                        lambda: bias_add_act(
                            sb_qT[:, c * CH : (c + 1) * CH], ps_q[:], sb_bq[:]
                        )
                    )
                return emit

            # emitted at END of B-iteration t (key); consumers:
            # kT chunk c needed by scores at t=4c; v group g by av_c0 per
            # av0_steps; qT c2/c3 by phase C.
            backlog = {
                0: mk_vgroup(0, nc.vector),
                1: mk_kchunk(1, nc.vector, 2),
                3: mk_vgroup(1, nc.vector),
                5: mk_kchunk(2, nc.vector, 6),
                6: mk_vgroup(2, nc.vector),
                8: mk_kchunk(3, nc.vector, 9),
                10: mk_qchunk(2, nc.vector, 11),
                12: mk_vgroup(3, nc.vector),
                13: mk_qchunk(3, nc.vector, 14),
            }

            def scores(t, h, qlo, ps):
                hd = slice(h * D, (h + 1) * D)
                for i in range(2):
                    nc.tensor.matmul(
                        ps[:, i * CH : (i + 1) * CH],
                        sb_kT[hd, t * 128 : (t + 1) * 128],
                        sb_qT[hd, qlo + i * CH : qlo + (i + 1) * CH],
                        start=True, stop=True,
                    )

            def av_step(slot, h, chalf, t, first, last):
                nc.tensor.matmul(
                    slot[:],
                    sb_v[:, t, h, :, :],
                    ex[h][:, t, chalf * CH : (chalf + 1) * CH],
                    start=first, stop=last,
                )

            RECIP_MAGIC = float(0x7EF127EA)
            i32 = mybir.dt.int32
            mult = mybir.AluOpType.mult
            addop = mybir.AluOpType.add

            def normalize(c, avx, heads=(0, 1)):
                # 1/Z on DVE (no ACT table swap): Schraudolph bit-trick
                # seed + 1 Newton step, then attnT[hd, chunk] = av_out*rz.
                # (divide is not an ISA op; GPSIMD cannot touch PSUM.)
                s_sl = slice(c * CH, (c + 1) * CH)
                for h in heads:
                    hd = slice(h * D, (h + 1) * D)
                    z_ap = avx[h][D : 2 * D, :]
                    rz = small.tile([D, CH], f32, tag="rz", name=f"rz{c}_{h}")
                    u = small.tile([D, CH], f32, tag="u", name=f"u{c}_{h}")
                    nc.vector.tensor_scalar(
                        rz[:].bitcast(i32), z_ap.bitcast(i32), -1, RECIP_MAGIC,
                        mult, addop,
                    )
                    for _ in range(1):
                        nc.vector.tensor_tensor(u[:], z_ap, rz[:], mult)
                        nc.vector.tensor_scalar(u[:], u[:], -1.0, 2.0, mult, addop)
                        nc.vector.tensor_tensor(rz[:], rz[:], u[:], mult)
                    nc.vector.tensor_mul(sb_attnT[hd, s_sl], avx[h][0:D, :], rz[:])

            def evict(eng, dst, src):
                if eng is nc.scalar:
                    eng.copy(dst, src)
                else:
                    eng.tensor_copy(dst, src)

            # out_proj for one 128-row s-tile: 2 half matmuls + evictions
            # + output DMA.  evict_engs/dma_eng chosen per call site.
            def out_tile(ts, evict_engs, dma_eng):
                sb_out = outp.tile([128, E], f16, tag="out", name=f"o{ts}")
                for ec in range(2):
                    ps_p = ps_a.tile([128, CH], f32, tag="a", name=f"op{ts}_{ec}")
                    nc.tensor.matmul(
                        ps_p[:],
                        sb_attnT[:, ts * 128 : (ts + 1) * 128],
                        sb_wout[:, ec * CH : (ec + 1) * CH],
                        start=True, stop=True,
                    )
                    evict(evict_engs[ec], sb_out[:, ec * CH : (ec + 1) * CH], ps_p[:])
                dma_eng.dma_start(
                    out=d_out[ts * 128 : (ts + 1) * 128, :], in_=sb_out[:]
                )

            # ---- phase B: scores p0 + exp; filler: backlog + AV c0 ----
            av0 = [ps_a.tile([128, CH], f32, tag="a", name=f"av0_{h}") for h in range(HC)]
            # av_c0 pacing: 16 steps over t=3..15 -> [2,1,1,1,1,1,1,1,1,1,1,2,2]
            av0_steps = {t: [] for t in range(NT)}
            j = 0
            for t in range(3, NT):
                n = 2 if t in (3, 14, 15) else 1
                for _ in range(n):
                    if j < NT:
                        av0_steps[t].append(j)
                        j += 1
            # scores run one t ahead of the exp stream: scores(t+1, h) is
            # emitted right after exp(t, h) so it starts the moment the
            # sc-ring slot frees — backlog/AV bursts never delay the exps.
            def mk_scores(phase, t, qlo):
                sc_ps = [
                    ps_sc.tile([128, W], f32, tag="sc", name=f"sc{phase}{t}_{h}")
                    for h in range(HC)
                ]
                for h in range(HC):
                    scores(t, h, qlo, sc_ps[h])
                return sc_ps

            # h0's exp runs exact on ACT; h1's runs as a one-pass
            # Schraudolph bf16 exp on DVE (bits = round(x*SCALE*128/ln2 +
            # 16251.5) read as bf16, ~2% sawtooth that largely cancels in
            # the softmax ratio) — halves the ACT stream, the old pacer.
            EXPA = float(SCALE * 128.0 / np.log(2.0))
            EXPB = 16251.5
            i16 = mybir.dt.int16

            def mk_exp(sc_ps, t):
                nc.scalar.activation(
                    ex[0][:, t, :], sc_ps[0][:],
                    mybir.ActivationFunctionType.Exp, scale=float(SCALE),
                )
                nc.vector.tensor_scalar(
                    ex[1][:, t, :].bitcast(i16), sc_ps[1][:], EXPA, EXPB,
                    mult, addop,
                )

            sc_cur = mk_scores("B", 0, 0)
            for t in range(NT):
                mk_exp(sc_cur, t)
                if t + 1 < NT:
                    sc_cur = mk_scores("B", t + 1, 0)
                for add in deferred_adds[t]:
                    add()
                for jj in av0_steps[t]:
                    for h in range(HC):
                        av_step(av0[h], h, 0, jj, jj == 0, jj == NT - 1)
                if t in backlog:
                    backlog[t]()
            for add in deferred_adds[NT]:
                add()

            # ---- B seam: normalize chunk c0 h0; h1's chain runs at C t=1
            # so it doesn't collide with the first offloaded exp ----
            normalize(0, av0, heads=(0,))

            # ---- phase C: scores p1 + exp; av1 t=0..3, av2 t>=4, av3 t>=9;
            #      out_proj c0 tiles t=1..4, c1 tiles t=5..8 ----
            av1 = [ps_a.tile([128, CH], f32, tag="a", name=f"av1_{h}") for h in range(HC)]
            av2 = None
            av3 = None
            # av1 consumes only B-phase ex values: run 4 steps/t for t=0..3
            # (stays ahead of the exp aliasing overwrite of ex[:, t, CH:W])
            av2_steps = {t: [] for t in range(NT)}
            j = 0
            for t in range(4, NT):
                n = {4: 2, 5: 2, 6: 2, 7: 2}.get(t, 1)
                for _ in range(n):
                    if j < NT:
                        av2_steps[t].append(j)
                        j += 1
            av3_steps = {t: [] for t in range(NT)}
            j = 0
            for t in range(10, NT):
                n = {15: 1}.get(t, 3)
                for _ in range(n):
                    if j < NT:
                        av3_steps[t].append(j)
                        j += 1
            # av1 j=0 must be read before exp_C(0) overwrites ex[:,0,:]
            # (Tile orders the WAR either way; emitting it first keeps the
            # exp from waiting long).
            for h in range(HC):
                av_step(av1[h], h, 1, 0, True, False)
            sc_cur = mk_scores("C", 0, W)
            for t in range(NT):
                mk_exp(sc_cur, t)
                if t + 1 < NT:
                    sc_cur = mk_scores("C", t + 1, W)
                if t < 4:
                    for jj in range(4 * t, 4 * t + 4):
                        if t == 0 and jj == 0:
                            continue
                        for h in range(HC):
                            av_step(av1[h], h, 1, jj, jj == 0, jj == NT - 1)
                if t == 1:
                    normalize(0, av0, heads=(1,))
                if t == 4:
                    # av1 complete: normalize chunk c1, freeing its banks
                    normalize(1, av1, heads=(0,))
                    av2 = [
                        ps_a.tile([128, CH], f32, tag="a", name=f"av2_{h}")
                        for h in range(HC)
                    ]
                if t == 5:
                    normalize(1, av1, heads=(1,))
                if t == 10:
                    av3 = [
                        ps_a.tile([128, CH], f32, tag="a", name=f"av3_{h}")
                        for h in range(HC)
                    ]
                for jj in av2_steps[t]:
                    for h in range(HC):
                        av_step(av2[h], h, 0, jj, jj == 0, jj == NT - 1)
                for jj in av3_steps[t]:
                    for h in range(HC):
                        av_step(av3[h], h, 1, jj, jj == 0, jj == NT - 1)
                # out_proj stream for the first-half chunks; ec0 eviction
                # rides ACT's slack (only h0 exps run there now)
                if 2 <= t <= 5:
                    out_tile(t - 2, (nc.scalar, nc.vector), nc.sync)
                elif 6 <= t <= 9:
                    out_tile(t - 2, (nc.scalar, nc.vector), nc.gpsimd)

            # ---- tail: c2/c3 normalize + out_proj + evict + DMA out ----
            # ACT is idle after the last exp: use table reciprocal there
            # (one table switch), muls on DVE.
            def act_recip(out_ap, in_ap):
                eng = nc.scalar
                inst = mybir.InstActivation(
                    name=nc.get_next_instruction_name(),
                    func=mybir.ActivationFunctionType.Reciprocal,
                    ins=[
                        eng.lower_ap(in_ap),
                        mybir.ImmediateValue(dtype=f32, value=0.0),
                        mybir.ImmediateValue(dtype=f32, value=1.0),
                        mybir.ImmediateValue(dtype=f32, value=0.0),
                    ],
                    outs=[eng.lower_ap(out_ap)],
                )
                eng.add_instruction(inst)

            for c, avx in ((2, av2), (3, av3)):
                s_sl = slice(c * CH, (c + 1) * CH)
                for h in range(HC):
                    hd = slice(h * D, (h + 1) * D)
                    rbc = small.tile([D, CH], f32, tag="rz", name=f"rbc{c}_{h}")
                    act_recip(rbc[:], avx[h][D : 2 * D, :])
                    nc.vector.tensor_mul(sb_attnT[hd, s_sl], avx[h][0:D, :], rbc[:])
            evict_rot = [
                (nc.scalar, nc.vector), (nc.vector, nc.scalar),
                (nc.scalar, nc.vector), (nc.vector, nc.scalar),
                (nc.scalar, nc.vector), (nc.vector, nc.scalar),
                (nc.scalar, nc.vector), (nc.vector, nc.scalar),
            ]
            dma_rot = [nc.sync, nc.gpsimd, nc.scalar, nc.sync,
                       nc.gpsimd, nc.scalar, nc.sync, nc.gpsimd]
            for i, ts in enumerate(range(8, 16)):
                out_tile(ts, evict_rot[i], dma_rot[i])

    nc.finalize()
    return nc


def _pack_w(w):
    # [E, J] -> [128, KE*J] in (p, k, m) order for a contiguous-row DMA
    return np.ascontiguousarray(
        np.asarray(w, np.float32).reshape(KE, 128, J).transpose(1, 0, 2).reshape(128, KE * J)
    ).astype(MM_NP)


def _prep_inputs(qkv, w_in, b_in, w_out):
    qkv2 = np.asarray(qkv, np.float32).reshape(S, E)
    qkvT = np.ascontiguousarray(qkv2.T).astype(MM_NP)
    w_in = np.asarray(w_in, np.float32)
    b_in = np.asarray(b_in, np.float32)
    w_out = np.asarray(w_out, np.float32)
    in_maps = []
    for c in range(NCORE):
        cols = slice(c * J, c * J + J)
        in_maps.append(
            {
                "qkvT": qkvT,
                "wq": _pack_w(w_in[:, :E][:, cols]),
                "wk": _pack_w(w_in[:, E : 2 * E][:, cols]),
                "wv": _pack_w(w_in[:, 2 * E :][:, cols]),
                "bq": np.ascontiguousarray(b_in[:E][cols]).reshape(J, 1),
                "bk": np.ascontiguousarray(b_in[E : 2 * E][cols]).reshape(J, 1),
                "bv": np.broadcast_to(
                    b_in[2 * E :][cols].reshape(1, J), (128, J)
                ).copy(),
                "wout": np.ascontiguousarray(w_out[cols, :]).astype(MM_NP),
            }
        )
    return in_maps


def kernel(qkv, w_in, b_in, w_out, b_out, _trace=False):
    global _cached
    if _cached is None:
        _cached = _build()
    nc = _cached
    in_maps = _prep_inputs(qkv, w_in, b_in, w_out)
    res = bass_utils.run_bass_kernel_spmd(
        nc, in_maps, core_ids=list(range(NCORE)), trace=_trace
    )
    acc = np.zeros((S, E), np.float64)
    for r in res.results:
        acc += r["partial"].astype(np.float64)
    out = (acc + np.asarray(b_out, np.float32)[None, :]).astype(np.float32)
    out = out.reshape(1, S, E)
    if _trace:
        kernel.last_exec_time_ns = res.exec_time_ns
    return out


# revision 60
# speedup vs baseline: 1.0610x; 1.0255x over previous
"""Multi-head self-attention (B=1, S=2048, E=1024, H=16, D=64) on 8 NeuronCores.

Tensor-parallel by head: core c owns heads {2c, 2c+1}.  v7 schedule:

  Phase A: 8 warm matmuls (HAM ramp); wk/wq + the first S-half of qkvT
    ride the two HW DGE queues (k-pair transfers); the second S-half is
    issued on gpsimd behind a data-gate so it never steals DMA bandwidth
    from the critical path; priority matmuls (kT c0, qT c0+c1) run
    k-outer as the pairs land.
  Phase B (t=0..15): scoresT(p0) per head (row-group concurrent K=64
    pairs), software-pipelined one t ahead of the exp stream so backlog
    bursts (rest of in_proj) never delay the exps.  h0's exp is exact on
    ACT; h1's is a one-pass Schraudolph bf16 exp on DVE (int16 bitcast,
    ~2% sawtooth that largely cancels in the softmax ratio).  AV c0
    trails.
  Phase C (t=0..15): scoresT(p1) + exp, same split.  AV c1 runs t=0..3
    (its exp data all exists at the seam); chunks c0/c1 are normalized
    in-phase (bit-trick + Newton 1/Z on DVE — divide is not an ISA op,
    ACT tables would thrash); out_proj tiles for c0/c1 stream through
    the phase with evictions + output DMA.  AV c2 from t=4, AV c3
    from t=10.
  Tail: only c2/c3: ACT table reciprocal (one switch), 8 out_proj
    tiles, evictions rotated over scalar/vector, output DMA striped.

Host sums the 8 partials and adds b_out.
"""

import os
import sys

import numpy as np

try:
    import concourse.bass as bass  # noqa: F401
except ImportError:
    sys.path.insert(0, "/opt/trn_rl_repo")

import ml_dtypes

import concourse.bass as bass
import concourse.mybir as mybir
import concourse.tile as tile
from concourse import bacc, bass_utils

S = 2048
E = 1024
H = 16
D = 64
NCORE = 8
HC = H // NCORE          # heads per core = 2
J = HC * D               # local feature width = 128
KE = E // 128            # contraction tiles for in_proj = 8
NT = S // 128            # 128-row tiles of the sequence = 16
CH = 512                 # s-chunk (one PSUM bank of f32)
NCH = S // CH            # s-chunks = 4
W = 2 * CH               # score tile width (2 chunks) = 1024
SCALE = 1.0 / np.sqrt(D)

MM_DT = mybir.dt.bfloat16
MM_NP = ml_dtypes.bfloat16

_cached = None


def _build():
    f32 = mybir.dt.float32
    f16 = mybir.dt.float16
    div = mybir.AluOpType.divide
    nc = bacc.Bacc("TRN2", target_bir_lowering=False, num_swdge_queues=4)

    d_qkvT = nc.dram_tensor("qkvT", (E, S), MM_DT, kind="ExternalInput")
    d_wq = nc.dram_tensor("wq", (128, KE * J), MM_DT, kind="ExternalInput")
    d_wk = nc.dram_tensor("wk", (128, KE * J), MM_DT, kind="ExternalInput")
    d_wv = nc.dram_tensor("wv", (128, KE * J), MM_DT, kind="ExternalInput")
    d_bq = nc.dram_tensor("bq", (J, 1), f32, kind="ExternalInput")
    d_bk = nc.dram_tensor("bk", (J, 1), f32, kind="ExternalInput")
    d_bv = nc.dram_tensor("bv", (128, J), f32, kind="ExternalInput")
    d_wout = nc.dram_tensor("wout", (J, E), MM_DT, kind="ExternalInput")
    d_out = nc.dram_tensor("partial", (S, E), f16, kind="ExternalOutput")

    dq = d_qkvT.rearrange("(k p) m -> p k m", p=128)

    with tile.TileContext(nc) as tc:
        with (
            tc.tile_pool(name="persist", bufs=1) as persist,
            tc.tile_pool(name="outp", bufs=8) as outp,
            tc.tile_pool(name="small", bufs=4) as small,
            # PSUM: 2 x [128,1024] (scores / qT pair)              = 4 banks
            #       4 x [128,512]  (AV + in_proj + out_proj accum) = 4 banks
            tc.tile_pool(name="ps_sc", bufs=2, space="PSUM") as ps_sc,
            tc.tile_pool(name="ps_a", bufs=4, space="PSUM") as ps_a,
        ):
            # ---- persistent SBUF ----
            sb_wq = persist.tile([128, KE, J], MM_DT)
            sb_wk = persist.tile([128, KE, J], MM_DT)
            sb_wv = persist.tile([128, KE, J], MM_DT)
            sb_bq = persist.tile([J, 1], f32)
            sb_bk = persist.tile([J, 1], f32)
            sb_bv = persist.tile([128, HC, D], f32)
            sb_bv2 = sb_bv
            sb_wout = persist.tile([J, E], MM_DT)
            sb_qkvT = persist.tile([128, KE, S], MM_DT)
            sb_qT = persist.tile([J, S], MM_DT)
            sb_kT = persist.tile([J, S], MM_DT)
            # v augmented per head with a 64-wide ones block: the AV matmul
            # then yields Z replicated on partitions 64..127
            sb_v = persist.tile([128, NT, HC, 2, D], MM_DT)
            sb_attnT = persist.tile([J, S], MM_DT)
            # exp buffers, one per head; C-phase aliases the same storage
            ex = [persist.tile([128, NT, W], MM_DT, name=f"ex{h}") for h in range(HC)]
            sb_warm = persist.tile([128, CH], MM_DT)

            # ---- t0: memsets (vector/gpsimd) ----
            nc.vector.memset(sb_warm[:], 0.125)
            nc.vector.memset(sb_v[:, :, 0, 1, :], 1.0)
            nc.gpsimd.memset(sb_v[:, :, 1, 1, :], 1.0)

            # ---- DMA issues ----
            # Everything the priority chain needs goes first (wk, wq, the
            # first S-half of qkvT) — descriptors of all in-flight
            # transfers share the 16 DMA engines, so the second half is
            # issued from the VECTOR queue *after* the priority bias adds:
            # it then starts streaming only once the first half is done
            # and never competes with the critical path.
            nc.sync.dma_start(out=sb_wk[:], in_=d_wk[:])
            nc.scalar.dma_start(out=sb_wq[:], in_=d_wq[:])
            hw2 = [nc.sync, nc.scalar]
            for kk in range(4):  # first half, k-pair granularity for gating
                hw2[kk % 2].dma_start(
                    out=sb_qkvT[:, 2 * kk : 2 * kk + 2, 0:1024],
                    in_=dq[:, 2 * kk : 2 * kk + 2, 0:1024],
                )
            nc.gpsimd.dma_start(out=sb_bq[:], in_=d_bq[:])
            nc.gpsimd.dma_start(out=sb_bk[:], in_=d_bk[:])
            nc.gpsimd.dma_start(out=sb_wv[:], in_=d_wv[:])
            nc.gpsimd.dma_start(out=sb_bv[:], in_=d_bv[:])

            # ---- phase A: priority in_proj (kT c0, qT c0+c1), k-outer,
            # warm fillers interleaved so HAM stays hot through the
            # DMA-gated waits.  Warm tiles live in the ps_a ring so they
            # don't cycle with q01 in the 2-deep sc ring. ----
            warm_ps = [
                ps_a.tile([128, CH], f32, tag="a", name=f"warm{i}")
                for i in range(2)
            ]

            def warm(n):
                for i in range(n):
                    nc.tensor.matmul(
                        warm_ps[i % 2][:], sb_warm[:, 0:128], sb_warm[:],
                        start=True, stop=True,
                    )

            warm(14)
            ps_kc0 = ps_a.tile([128, CH], f32, tag="a", name="kc0")
            ps_q01 = ps_sc.tile([128, W], f32, tag="sc", name="q01")
            for k in range(KE):
                nc.tensor.matmul(
                    ps_kc0[:], sb_wk[:, k, :], sb_qkvT[:, k, 0:CH],
                    start=(k == 0), stop=(k == KE - 1),
                )
                nc.tensor.matmul(
                    ps_q01[:, 0:CH], sb_wq[:, k, :], sb_qkvT[:, k, 0:CH],
                    start=(k == 0), stop=(k == KE - 1),
                )
                nc.tensor.matmul(
                    ps_q01[:, CH:W], sb_wq[:, k, :], sb_qkvT[:, k, CH:W],
                    start=(k == 0), stop=(k == KE - 1),
                )

            # kc0's bias add on ACT (activation with per-partition bias,
            # identity is in every table), parallel with q01's on DVE
            def bias_add_act(out, in_, bias):
                nc.scalar.activation(
                    out, in_, mybir.ActivationFunctionType.Identity,
                    bias=bias, scale=1.0,
                )

            bias_add_act(sb_kT[:, 0:CH], ps_kc0[:], sb_bk[:])
            nc.vector.tensor_scalar_add(sb_qT[:, 0:W], ps_q01[:], sb_bq[:])
            # second S-half + wout: issued on gpsimd BEHIND a tiny copy
            # that depends on the priority-add output, so these transfers
            # only start streaming once the first half has landed and
            # never steal DMA bandwidth from the critical path.
            nc.gpsimd.tensor_copy(sb_warm[0:1, 0:1], sb_qT[0:1, 0:1])
            nc.gpsimd.dma_start(
                out=sb_qkvT[:, 0:4, 1024:2048], in_=dq[:, 0:4, 1024:2048]
            )
            nc.gpsimd.dma_start(
                out=sb_qkvT[:, 4:8, 1024:2048], in_=dq[:, 4:8, 1024:2048]
            )
            nc.gpsimd.dma_start(out=sb_wout[:], in_=d_wout[:])

            # ---- backlog filler units for phase B ----
            def mk_vgroup(g, eng):
                def emit():
                    ps_v = ps_a.tile([128, 4, HC, D], f32, tag="a", name=f"v{g}")
                    for ti in range(4):
                        t = 4 * g + ti
                        for k in range(KE):
                            nc.tensor.matmul(
                                ps_v[:, ti, :, :],
                                sb_qkvT[:, k, t * 128 : (t + 1) * 128],
                                sb_wv[:, k, :],
                                start=(k == 0), stop=(k == KE - 1),
                            )
                    # add bias into sb_v (v cols only, skip the ones
                    # blocks) — one strided add per t-tile
                    for ti in range(4):
                        t = 4 * g + ti
                        eng.tensor_add(
                            sb_v[:, t, :, 0, :], ps_v[:, ti, :, :], sb_bv2[:]
                        )
                return emit

            # kchunk/qchunk adds run on ACT (slack there), DEFERRED one
            # iteration so they never head-of-line-block the exp stream
            # while their matmul accumulation is in flight.
            deferred_adds = {t: [] for t in range(NT + 1)}

            def mk_kchunk(c, eng, t_add):
                def emit():
                    ps_k = ps_a.tile([128, CH], f32, tag="a", name=f"kc{c}")
                    for k in range(KE):
                        nc.tensor.matmul(
                            ps_k[:], sb_wk[:, k, :],
                            sb_qkvT[:, k, c * CH : (c + 1) * CH],
                            start=(k == 0), stop=(k == KE - 1),
                        )
                    deferred_adds[t_add].append(
                        lambda: bias_add_act(
                            sb_kT[:, c * CH : (c + 1) * CH], ps_k[:], sb_bk[:]
                        )
                    )
                return emit

            def mk_qchunk(c, eng, t_add):
                def emit():
                    ps_q = ps_a.tile([128, CH], f32, tag="a", name=f"qc{c}")
                    for k in range(KE):
                        nc.tensor.matmul(
                            ps_q[:], sb_wq[:, k, :],
                            sb_qkvT[:, k, c * CH : (c + 1) * CH],
                            start=(k == 0), stop=(k == KE - 1),
                        )
                    deferred_adds[t_add].append(
                        lambda: bias_add_act(
                            sb_qT[:, c * CH : (c + 1) * CH], ps_q[:], sb_bq[:]
                        )
                    )
                return emit

            # emitted at END of B-iteration t (key); consumers:
            # kT chunk c needed by scores at t=4c; v group g by av_c0 per
            # av0_steps; qT c2/c3 by phase C.
            backlog = {
                0: mk_vgroup(0, nc.vector),
                2: mk_kchunk(1, nc.vector, 3),
                4: mk_vgroup(1, nc.vector),
                5: mk_kchunk(2, nc.vector, 6),
                6: mk_vgroup(2, nc.vector),
                9: mk_kchunk(3, nc.vector, 10),
                10: mk_qchunk(2, nc.vector, 11),
                12: mk_vgroup(3, nc.vector),
                13: mk_qchunk(3, nc.vector, 14),
            }

            def scores(t, h, qlo, ps):
                hd = slice(h * D, (h + 1) * D)
                for i in range(2):
                    nc.tensor.matmul(
                        ps[:, i * CH : (i + 1) * CH],
                        sb_kT[hd, t * 128 : (t + 1) * 128],
                        sb_qT[hd, qlo + i * CH : qlo + (i + 1) * CH],
                        start=True, stop=True,
                    )

            def av_step(slot, h, chalf, t, first, last):
                nc.tensor.matmul(
                    slot[:],
                    sb_v[:, t, h, :, :],
                    ex[h][:, t, chalf * CH : (chalf + 1) * CH],
                    start=first, stop=last,
                )

            RECIP_MAGIC = float(0x7EF127EA)
            i32 = mybir.dt.int32
            mult = mybir.AluOpType.mult
            addop = mybir.AluOpType.add

            def normalize(c, avx, heads=(0, 1)):
                # 1/Z on DVE (no ACT table swap): Schraudolph bit-trick
                # seed + 1 Newton step, then attnT[hd, chunk] = av_out*rz.
                # (divide is not an ISA op; GPSIMD cannot touch PSUM.)
                s_sl = slice(c * CH, (c + 1) * CH)
                for h in heads:
                    hd = slice(h * D, (h + 1) * D)
                    z_ap = avx[h][D : 2 * D, :]
                    rz = small.tile([D, CH], f32, tag="rz", name=f"rz{c}_{h}")
                    u = small.tile([D, CH], f32, tag="u", name=f"u{c}_{h}")
                    nc.vector.tensor_scalar(
                        rz[:].bitcast(i32), z_ap.bitcast(i32), -1, RECIP_MAGIC,
                        mult, addop,
                    )
                    for _ in range(1):
                        nc.vector.tensor_tensor(u[:], z_ap, rz[:], mult)
                        nc.vector.tensor_scalar(u[:], u[:], -1.0, 2.0, mult, addop)
                        nc.vector.tensor_tensor(rz[:], rz[:], u[:], mult)
                    nc.vector.tensor_mul(sb_attnT[hd, s_sl], avx[h][0:D, :], rz[:])

            def evict(eng, dst, src):
                if eng is nc.scalar:
                    eng.copy(dst, src)
                else:
                    eng.tensor_copy(dst, src)

            # out_proj for one 128-row s-tile: 2 half matmuls + evictions
            # + output DMA.  evict_engs/dma_eng chosen per call site.
            def out_tile(ts, evict_engs, dma_eng):
                sb_out = outp.tile([128, E], f16, tag="out", name=f"o{ts}")
                for ec in range(2):
                    ps_p = ps_a.tile([128, CH], f32, tag="a", name=f"op{ts}_{ec}")
                    nc.tensor.matmul(
                        ps_p[:],
                        sb_attnT[:, ts * 128 : (ts + 1) * 128],
                        sb_wout[:, ec * CH : (ec + 1) * CH],
                        start=True, stop=True,
                    )
                    evict(evict_engs[ec], sb_out[:, ec * CH : (ec + 1) * CH], ps_p[:])
                dma_eng.dma_start(
                    out=d_out[ts * 128 : (ts + 1) * 128, :], in_=sb_out[:]
                )

            # ---- phase B: scores p0 + exp; filler: backlog + AV c0 ----
            av0 = [ps_a.tile([128, CH], f32, tag="a", name=f"av0_{h}") for h in range(HC)]
            # av_c0 pacing: 16 steps over t=3..15 -> [2,1,1,1,1,1,1,1,1,1,1,2,2]
            av0_steps = {t: [] for t in range(NT)}
            j = 0
            for t in range(3, NT):
                n = 2 if t in (3, 14, 15) else 1
                for _ in range(n):
                    if j < NT:
                        av0_steps[t].append(j)
                        j += 1
            # scores run one t ahead of the exp stream: scores(t+1, h) is
            # emitted right after exp(t, h) so it starts the moment the
            # sc-ring slot frees — backlog/AV bursts never delay the exps.
            def mk_scores(phase, t, qlo):
                sc_ps = [
                    ps_sc.tile([128, W], f32, tag="sc", name=f"sc{phase}{t}_{h}")
                    for h in range(HC)
                ]
                for h in range(HC):
                    scores(t, h, qlo, sc_ps[h])
                return sc_ps

            # h0's exp runs exact on ACT; h1's runs as a one-pass
            # Schraudolph bf16 exp on DVE (bits = round(x*SCALE*128/ln2 +
            # 16251.5) read as bf16, ~2% sawtooth that largely cancels in
            # the softmax ratio) — halves the ACT stream, the old pacer.
            EXPA = float(SCALE * 128.0 / np.log(2.0))
            EXPB = 16251.5
            i16 = mybir.dt.int16

            def mk_exp(sc_ps, t):
                nc.scalar.activation(
                    ex[0][:, t, :], sc_ps[0][:],
                    mybir.ActivationFunctionType.Exp, scale=float(SCALE),
                )
                nc.vector.tensor_scalar(
                    ex[1][:, t, :].bitcast(i16), sc_ps[1][:], EXPA, EXPB,
                    mult, addop,
                )

            sc_cur = mk_scores("B", 0, 0)
            for t in range(NT):
                mk_exp(sc_cur, t)
                for add in deferred_adds[t]:
                    add()
                if t + 1 < NT:
                    sc_cur = mk_scores("B", t + 1, 0)
                for jj in av0_steps[t]:
                    for h in range(HC):
                        av_step(av0[h], h, 0, jj, jj == 0, jj == NT - 1)
                if t in backlog:
                    backlog[t]()

            # ---- B seam: normalize chunk c0 h0; h1's chain runs at C t=1
            # so it doesn't collide with the first offloaded exp ----
            normalize(0, av0, heads=(0,))

            # ---- phase C: scores p1 + exp; av1 t=0..3, av2 t>=4, av3 t>=9;
            #      out_proj c0 tiles t=1..4, c1 tiles t=5..8 ----
            av1 = [ps_a.tile([128, CH], f32, tag="a", name=f"av1_{h}") for h in range(HC)]
            av2 = None
            av3 = None
            # av1 consumes only B-phase ex values: run 4 steps/t for t=0..3
            # (stays ahead of the exp aliasing overwrite of ex[:, t, CH:W])
            av2_steps = {t: [] for t in range(NT)}
            j = 0
            for t in range(4, NT):
                n = {4: 2, 5: 2, 6: 2, 7: 2}.get(t, 1)
                for _ in range(n):
                    if j < NT:
                        av2_steps[t].append(j)
                        j += 1
            av3_steps = {t: [] for t in range(NT)}
            j = 0
            for t in range(10, NT):
                n = {15: 1}.get(t, 3)
                for _ in range(n):
                    if j < NT:
                        av3_steps[t].append(j)
                        j += 1
            # av1 j=0 must be read before exp_C(0) overwrites ex[:,0,:]
            # (Tile orders the WAR either way; emitting it first keeps the
            # exp from waiting long).
            for h in range(HC):
                av_step(av1[h], h, 1, 0, True, False)
            sc_cur = mk_scores("C", 0, W)
            for t in range(NT):
                mk_exp(sc_cur, t)
                if t + 1 < NT:
                    sc_cur = mk_scores("C", t + 1, W)
                if t < 4:
                    for jj in range(4 * t, 4 * t + 4):
                        if t == 0 and jj == 0:
                            continue
                        for h in range(HC):
                            av_step(av1[h], h, 1, jj, jj == 0, jj == NT - 1)
                if t == 1:
                    normalize(0, av0, heads=(1,))
                if t == 4:
                    # av1 complete: normalize chunk c1, freeing its banks
                    normalize(1, av1, heads=(0,))
                    av2 = [
                        ps_a.tile([128, CH], f32, tag="a", name=f"av2_{h}")
                        for h in range(HC)
                    ]
                if t == 5:
                    normalize(1, av1, heads=(1,))
                if t == 10:
                    av3 = [
                        ps_a.tile([128, CH], f32, tag="a", name=f"av3_{h}")
                        for h in range(HC)
                    ]
                for jj in av2_steps[t]:
                    for h in range(HC):
                        av_step(av2[h], h, 0, jj, jj == 0, jj == NT - 1)
                for jj in av3_steps[t]:
                    for h in range(HC):
                        av_step(av3[h], h, 1, jj, jj == 0, jj == NT - 1)
                # out_proj stream for the first-half chunks; ec0 eviction
                # rides ACT's slack (only h0 exps run there now)
                if 2 <= t <= 5:
                    out_tile(t - 2, (nc.scalar, nc.vector), nc.sync)
                elif 6 <= t <= 9:
                    out_tile(t - 2, (nc.scalar, nc.vector), nc.gpsimd)

            # ---- tail: c2/c3 normalize + out_proj + evict + DMA out ----
            # ACT is idle after the last exp: use table reciprocal there
            # (one table switch), muls on DVE.
            def act_recip(out_ap, in_ap):
                eng = nc.scalar
                inst = mybir.InstActivation(
                    name=nc.get_next_instruction_name(),
                    func=mybir.ActivationFunctionType.Reciprocal,
                    ins=[
                        eng.lower_ap(in_ap),
                        mybir.ImmediateValue(dtype=f32, value=0.0),
                        mybir.ImmediateValue(dtype=f32, value=1.0),
                        mybir.ImmediateValue(dtype=f32, value=0.0),
                    ],
                    outs=[eng.lower_ap(out_ap)],
                )
                eng.add_instruction(inst)

            for c, avx in ((2, av2), (3, av3)):
                s_sl = slice(c * CH, (c + 1) * CH)
                for h in range(HC):
                    hd = slice(h * D, (h + 1) * D)
                    rbc = small.tile([D, CH], f32, tag="rz", name=f"rbc{c}_{h}")
                    act_recip(rbc[:], avx[h][D : 2 * D, :])
                    nc.vector.tensor_mul(sb_attnT[hd, s_sl], avx[h][0:D, :], rbc[:])
            evict_rot = [
                (nc.scalar, nc.vector), (nc.vector, nc.scalar),
                (nc.scalar, nc.vector), (nc.vector, nc.scalar),
                (nc.scalar, nc.vector), (nc.vector, nc.scalar),
                (nc.scalar, nc.vector), (nc.vector, nc.scalar),
            ]
            dma_rot = [nc.sync, nc.gpsimd, nc.scalar, nc.sync,
                       nc.gpsimd, nc.scalar, nc.sync, nc.gpsimd]
            for i, ts in enumerate(range(8, 16)):
                out_tile(ts, evict_rot[i], dma_rot[i])

    nc.finalize()
    return nc


def _pack_w(w):
    # [E, J] -> [128, KE*J] in (p, k, m) order for a contiguous-row DMA
    return np.ascontiguousarray(
        np.asarray(w, np.float32).reshape(KE, 128, J).transpose(1, 0, 2).reshape(128, KE * J)
    ).astype(MM_NP)


def _prep_inputs(qkv, w_in, b_in, w_out):
    qkv2 = np.asarray(qkv, np.float32).reshape(S, E)
    qkvT = np.ascontiguousarray(qkv2.T).astype(MM_NP)
    w_in = np.asarray(w_in, np.float32)
    b_in = np.asarray(b_in, np.float32)
    w_out = np.asarray(w_out, np.float32)
    in_maps = []
    for c in range(NCORE):
        cols = slice(c * J, c * J + J)
        in_maps.append(
            {
                "qkvT": qkvT,
                "wq": _pack_w(w_in[:, :E][:, cols]),
                "wk": _pack_w(w_in[:, E : 2 * E][:, cols]),
                "wv": _pack_w(w_in[:, 2 * E :][:, cols]),
                "bq": np.ascontiguousarray(b_in[:E][cols]).reshape(J, 1),
                "bk": np.ascontiguousarray(b_in[E : 2 * E][cols]).reshape(J, 1),
                "bv": np.broadcast_to(
                    b_in[2 * E :][cols].reshape(1, J), (128, J)
                ).copy(),
                "wout": np.ascontiguousarray(w_out[cols, :]).astype(MM_NP),
            }
        )
    return in_maps


def kernel(qkv, w_in, b_in, w_out, b_out, _trace=False):
    global _cached
    if _cached is None:
        _cached = _build()
    nc = _cached
    in_maps = _prep_inputs(qkv, w_in, b_in, w_out)
    res = bass_utils.run_bass_kernel_spmd(
        nc, in_maps, core_ids=list(range(NCORE)), trace=_trace
    )
    acc = np.zeros((S, E), np.float64)
    for r in res.results:
        acc += r["partial"].astype(np.float64)
    out = (acc + np.asarray(b_out, np.float32)[None, :]).astype(np.float32)
    out = out.reshape(1, S, E)
    if _trace:
        kernel.last_exec_time_ns = res.exec_time_ns
    return out
